# revision 28
# baseline (speedup 1.0000x reference)
"""Trainium2 Bass kernel for batched multi-head attention (no scale).

Problem: q,k,v [B=4, H=16, S=2048, D=128] fp32;
    out = softmax(q @ k^T) @ v   (no 1/sqrt(D) scaling)

Sharding: B*H = 64 heads, 8 heads per core across 8 NeuronCores.

Per-head device algorithm (layout: S^T tiles [kk, q]):
  S^T[kk, q]  = matmul(lhsT=K^T fp16, rhs=Q^T fp16)    -> PSUM f32
  P[kk, q]    = exp(S^T - 64) -> bf16 SBUF
      6/8 groups: ScalarE activation (exact exp)
      2/8 groups: DVE Schraudolph fast-exp: one tensor_scalar computing
                  round(A*s + B) with uint16 saturating convert, bitcast
                  to bf16.  A = 128/ln2, B folds the -64 bias and the
                  mantissa-interpolation centering constant.  Negative
                  bitpatterns (P < ~1e-38) saturate to 0 = +0.0.
  out^T[d, q]+= matmul(lhsT=V fp16, rhs=P bf16)         (PSUM acc)
  l[q]       += matmul(lhsT=ones, rhs=P) 4-way col-tiled (PSUM acc)
                (the 4 strip matmuls run concurrently in disjoint PE
                column quadrants; two 4-strip bursts are emitted back
                to back to minimize full-width pipeline refills)

The whole kernel is ONE software pipeline over global group index gg
(8 heads x 4 qtiles x 8 groups = 256 steps): QK(gg) + exp(gg) issue at
step gg, AV(gg-3) behind them, the paired l-bursts for groups
(gg-7..gg-4) after that, and a q-tile's PSUM->SBUF copies + DMA-out at
gg-12 once its last AV and l-burst have issued.  No per-qtile or
per-head pipeline drain: the PE stream is uniform from first to last
group (measured ~278us busy vs the 272us QK+AV+l streaming floor at
2.4GHz; exp splits 6/8 ScalarE + 2/8 DVE so neither stalls the PE).

Host pre-transposes Q,K to [D,S] fp16, pre-permutes V to partition-major
[128, NKB, 128] fp16 (so its DMA is linear), and post-applies
out = (out^T / l)^T.
"""

import os

import numpy as np

import concourse.bass as bass
import concourse.tile as tile
from concourse import bacc, mybir
from concourse.bass_utils import run_bass_kernel_spmd

B, H, S, D = 4, 16, 2048, 128
N_CORES = 8
HPC = (B * H) // N_CORES  # heads per core
QT = 512                  # q-tile width (one fp32 PSUM bank)
NQT = S // QT             # 4 q tiles per head
KB = 128                  # kk block (contraction of one matmul)
NKB = S // KB             # 16 kk blocks
GEXP = 2                  # kk blocks batched per exp instruction
NG = NKB // GEXP          # 8 groups per q tile
GPQ = NG                  # groups per q tile
GPH = NQT * NG            # groups per head
TOT = HPC * GPH           # global group count
DVE_GROUPS = (2, 5)       # groups (mod NG) whose exp runs on DVE
EXP_BIAS = -64.0
SCH_A = 128.0 / float(np.log(2.0))          # 184.664...
SCH_B = 16256.0 - 5.5 + EXP_BIAS * SCH_A    # fold bias; -5.5 centers err
F32 = mybir.dt.float32
BF16 = mybir.dt.bfloat16
FP16 = mybir.dt.float16
U16 = mybir.dt.uint16

_NC_CACHE = None


def _build_nc():
    nc = bacc.Bacc("TRN2", target_bir_lowering=False, debug=False)

    qT_d = nc.dram_tensor("qT", [HPC, D, S], FP16, kind="ExternalInput")
    kT_d = nc.dram_tensor("kT", [HPC, D, S], FP16, kind="ExternalInput")
    v_d = nc.dram_tensor("v", [HPC, 128, NKB, D], FP16, kind="ExternalInput")
    oT_d = nc.dram_tensor("outT", [HPC, D, S], F32, kind="ExternalOutput")
    l_d = nc.dram_tensor("lsum", [HPC, NQT, 4, QT], F32, kind="ExternalOutput")

    with tile.TileContext(nc) as tc:
        with (
            tc.tile_pool(name="io", bufs=3) as io,
            tc.tile_pool(name="pexp", bufs=10) as pexp,
            tc.tile_pool(name="osb", bufs=3) as osb_pool,
            tc.tile_pool(name="small", bufs=1) as small,
            tc.tile_pool(name="st", bufs=2, space="PSUM") as st_pool,
            tc.tile_pool(name="acc", bufs=2, space="PSUM") as acc_pool,
        ):
            ones_sb = small.tile([128, 1], BF16)
            nc.vector.memset(ones_sb[:], 1.0)
            bias_sb = small.tile([128, 1], F32)
            nc.vector.memset(bias_sb[:], EXP_BIAS)
            wu_sb = small.tile([128, 256], FP16)
            nc.vector.memset(wu_sb[:], 0.0)
            scr_sb = small.tile([128, 256], BF16)

            # PE pstate warmup while the first DMAs stream in; the dummy
            # activation preloads the exp table off the critical path.
            wu_ps = st_pool.tile([128, GEXP * QT], F32, tag="st")
            nc.tensor.matmul(
                wu_ps[:, :256], wu_sb[:, :128], wu_sb[:],
                start=True, stop=True,
            )
            nc.scalar.activation(
                scr_sb[:],
                wu_sb[:],
                mybir.ActivationFunctionType.Exp,
                bias=bias_sb[:, :],
                scale=1.0,
            )
            for _ in range(10):
                nc.tensor.matmul(
                    wu_ps[:, :256], wu_sb[:, :128], wu_sb[:],
                    start=True, stop=True,
                )

            heads = {}   # hd -> (qT_sb, kT_sb, v_sb)
            accs = {}    # qt_start_gg -> (out_ps, l_ps)
            p_tiles = {} # gg -> p_sb

            for gg in range(TOT + 8):
                # AV for group gg-3
                av = gg - 3
                if 0 <= av < TOT:
                    hd2, rem2 = divmod(av, GPH)
                    g2 = rem2 % GPQ
                    out_ps = accs[av - g2][0]
                    v_sb2 = heads[hd2][2]
                    p_sb2 = p_tiles[av]
                    for j in range(GEXP):
                        kb = g2 * GEXP + j
                        nc.tensor.matmul(
                            out_ps[:],
                            v_sb2[:, kb, :],
                            p_sb2[:, j * QT:(j + 1) * QT],
                            start=(kb == 0),
                            stop=(kb == NKB - 1),
                        )

                if gg < TOT:
                    hd, rem = divmod(gg, GPH)
                    qt, g = divmod(rem, GPQ)

                    if rem == 0:
                        qT_sb = io.tile([128, S], FP16, tag="qT")
                        kT_sb = io.tile([128, S], FP16, tag="kT")
                        v_sb = io.tile([128, NKB, D], FP16, tag="v")
                        heads[hd] = (qT_sb, kT_sb, v_sb)
                        if hd != 0:
                            nc.gpsimd.dma_start(out=v_sb[:], in_=v_d[hd])
                        if hd == 0:
                            # chunked across three queues so QK starts
                            # early and kT keeps ahead of the QK stream
                            nc.sync.dma_start(
                                out=kT_sb[:, :256], in_=kT_d[0, :, :256])
                            nc.scalar.dma_start(
                                out=qT_sb[:, :QT], in_=qT_d[0, :, :QT])
                            nc.sync.dma_start(
                                out=kT_sb[:, 256:QT], in_=kT_d[0, :, 256:QT])
                            nc.gpsimd.dma_start(
                                out=kT_sb[:, QT:2 * QT],
                                in_=kT_d[0, :, QT:2 * QT])
                            nc.sync.dma_start(
                                out=kT_sb[:, 2 * QT:3 * QT],
                                in_=kT_d[0, :, 2 * QT:3 * QT])
                            nc.scalar.dma_start(
                                out=qT_sb[:, QT:], in_=qT_d[0, :, QT:])
                            nc.sync.dma_start(
                                out=kT_sb[:, 3 * QT:], in_=kT_d[0, :, 3 * QT:])
                            nc.gpsimd.dma_start(
                                out=v_sb[:, :4, :], in_=v_d[0, :, :4, :])
                            nc.gpsimd.dma_start(
                                out=v_sb[:, 4:, :], in_=v_d[0, :, 4:, :])
                        else:
                            nc.sync.dma_start(out=qT_sb[:], in_=qT_d[hd])
                            nc.sync.dma_start(out=kT_sb[:], in_=kT_d[hd])
                    else:
                        qT_sb, kT_sb, v_sb = heads[hd]

                    if g == 0:
                        out_ps_new = acc_pool.tile([128, QT], F32, tag="out")
                        l_ps_new = acc_pool.tile([128, QT], F32, tag="l")
                        accs[gg] = (out_ps_new, l_ps_new)

                    # QK for group gg
                    q_sl = qT_sb[:, qt * QT:(qt + 1) * QT]
                    st_ps = st_pool.tile([128, GEXP * QT], F32, tag="st")
                    for j in range(GEXP):
                        kb = g * GEXP + j
                        nc.tensor.matmul(
                            st_ps[:, j * QT:(j + 1) * QT],
                            kT_sb[:, kb * KB:(kb + 1) * KB],
                            q_sl,
                            start=True,
                            stop=True,
                        )
                    # exp for group gg
                    p_sb = pexp.tile([128, GEXP * QT], BF16, tag="p")
                    if gg >= TOT - 2:
                        nc.scalar.activation(
                            p_sb[:, :QT],
                            st_ps[:, :QT],
                            mybir.ActivationFunctionType.Exp,
                            bias=bias_sb[:, :],
                            scale=1.0,
                        )
                        nc.vector.tensor_scalar(
                            p_sb[:, QT:].bitcast(U16),
                            st_ps[:, QT:],
                            SCH_A,
                            SCH_B,
                            mybir.AluOpType.mult,
                            mybir.AluOpType.add,
                        )
                    elif g in DVE_GROUPS:
                        nc.vector.tensor_scalar(
                            p_sb[:].bitcast(U16),
                            st_ps[:],
                            SCH_A,
                            SCH_B,
                            mybir.AluOpType.mult,
                            mybir.AluOpType.add,
                        )
                    else:
                        nc.scalar.activation(
                            p_sb[:],
                            st_ps[:],
                            mybir.ActivationFunctionType.Exp,
                            bias=bias_sb[:, :],
                            scale=1.0,
                        )
                    p_tiles[gg] = p_sb

                # paired l-bursts for groups (gg-7 .. gg-4): two 4-strip
                # bursts back to back halve the burst->QK pipeline refills
                lb = gg - 7
                if lb >= 0 and lb % 4 == 0 and lb < TOT:
                    g3 = lb % GPQ
                    l_ps = accs[lb - g3][1]
                    for half in range(2):
                        r = g3 // 2 + half
                        for j4 in range(4):
                            psrc = p_tiles[lb + 2 * half + j4 // GEXP]
                            nc.tensor.matmul(
                                l_ps[32 * j4:32 * j4 + 1, :],
                                ones_sb[:],
                                psrc[:, (j4 % GEXP) * QT:(j4 % GEXP + 1) * QT],
                                start=(r == 0),
                                stop=(r == NG // 2 - 1),
                                tile_position=(0, 32 * j4),
                            )

                # copies + DMA out for the q tile whose last AV (step
                # qs+10) and last l-burst (step qs+11) have now issued;
                # +14 keeps the copies behind both DVE exps of the
                # following q tile on the in-order DVE queue
                qs = gg - 14
                if qs >= 0 and qs % GPQ == 0:
                    hd4, rem4 = divmod(qs, GPH)
                    qt4 = rem4 // GPQ
                    out_ps, l_ps = accs.pop(qs)
                    out_sb = osb_pool.tile([128, QT], F32, tag="osb")
                    l_sb = osb_pool.tile([128, QT], F32, tag="lsb")
                    # alternate output DMA queues to halve final flush
                    eng_a = nc.gpsimd if qt4 % 2 == 0 else nc.sync
                    eng_b = nc.sync if qt4 % 2 == 0 else nc.gpsimd
                    if qs == TOT - GPQ:
                        # last q tile: chunk copy+DMA to shorten the drain
                        hq = QT // 2
                        nc.vector.tensor_copy(out_sb[:, :hq], out_ps[:, :hq])
                        eng_a.dma_start(
                            out=oT_d[hd4, :, qt4 * QT:qt4 * QT + hq],
                            in_=out_sb[:, :hq],
                        )
                        nc.vector.tensor_copy(out_sb[:, hq:], out_ps[:, hq:])
                        eng_b.dma_start(
                            out=oT_d[hd4, :, qt4 * QT + hq:(qt4 + 1) * QT],
                            in_=out_sb[:, hq:],
                        )
                        nc.vector.tensor_copy(l_sb[:], l_ps[:])
                        eng_a.dma_start(
                            out=l_d[hd4, qt4], in_=l_sb[0:128:32, :]
                        )
                    else:
                        nc.vector.tensor_copy(out_sb[:], out_ps[:])
                        nc.vector.tensor_copy(l_sb[:], l_ps[:])
                        eng_a.dma_start(
                            out=oT_d[hd4, :, qt4 * QT:(qt4 + 1) * QT],
                            in_=out_sb[:],
                        )
                        eng_b.dma_start(
                            out=l_d[hd4, qt4], in_=l_sb[0:128:32, :]
                        )

                if gg - 8 in p_tiles:
                    del p_tiles[gg - 8]
    nc.finalize()
    return nc


def _get_nc():
    global _NC_CACHE
    if _NC_CACHE is None:
        _NC_CACHE = _build_nc()
    return _NC_CACHE


def kernel(q, k, v):
    q = np.asarray(q, dtype=np.float32).reshape(B * H, S, D)
    k = np.asarray(k, dtype=np.float32).reshape(B * H, S, D)
    v = np.asarray(v, dtype=np.float32).reshape(B * H, S, D)

    in_maps = []
    for c in range(N_CORES):
        sl = slice(c * HPC, (c + 1) * HPC)
        # v: [HPC, S, D] -> partition-major [HPC, 128, NKB, D]
        vperm = v[sl].reshape(HPC, NKB, 128, D).transpose(0, 2, 1, 3)
        in_maps.append(
            {
                "qT": np.ascontiguousarray(
                    q[sl].transpose(0, 2, 1)).astype(np.float16),
                "kT": np.ascontiguousarray(
                    k[sl].transpose(0, 2, 1)).astype(np.float16),
                "v": np.ascontiguousarray(vperm).astype(np.float16),
            }
        )

    nc = _get_nc()
    trace = bool(int(os.environ.get("KERNEL_TRACE", "0")))
    res = run_bass_kernel_spmd(
        nc, in_maps, core_ids=list(range(N_CORES)), trace=trace
    )
    if trace:
        print(f"HW exec time: {res.exec_time_ns} ns")
        if res.instructions_and_trace:
            print(f"Trace: {res.instructions_and_trace[1]}")

    out = np.empty((B * H, S, D), dtype=np.float32)
    for c in range(N_CORES):
        oT = res.results[c]["outT"]  # [HPC, D, S]
        l = res.results[c]["lsum"].sum(axis=2).reshape(HPC, S)  # fold strips
        out[c * HPC:(c + 1) * HPC] = oT.transpose(0, 2, 1) / l[:, :, None]
    return out.reshape(B, H, S, D)


# revision 29
# speedup vs baseline: 1.0882x; 1.0882x over previous
"""Trainium2 Bass kernel for batched multi-head attention (no scale).

Problem: q,k,v [B=4, H=16, S=2048, D=128] fp32;
    out = softmax(q @ k^T) @ v   (no 1/sqrt(D) scaling)

Sharding: B*H = 64 heads, 8 heads per core across 8 NeuronCores.

Per-head device algorithm (layout: S^T tiles [kk, q]):
  S^T[kk, q]  = matmul(lhsT=K^T fp16, rhs=Q^T fp16)    -> PSUM f32
  P[kk, q]    = exp(S^T - 64) -> bf16 SBUF
      6/8 groups: ScalarE activation (exact exp)
      2/8 groups: DVE Schraudolph fast-exp: one tensor_scalar computing
                  round(A*s + B) with uint16 saturating convert, bitcast
                  to bf16.  A = 128/ln2, B folds the -64 bias and the
                  mantissa-interpolation centering constant.  Negative
                  bitpatterns (P < ~1e-38) saturate to 0 = +0.0.
  out^T[d, q]+= matmul(lhsT=V fp16, rhs=P bf16)         (PSUM acc)
  l[q]       += matmul(lhsT=ones, rhs=P) 4-way col-tiled (PSUM acc)
                (the 4 strip matmuls run concurrently in disjoint PE
                column quadrants; two 4-strip bursts are emitted back
                to back to minimize full-width pipeline refills)

The whole kernel is ONE software pipeline over global group index gg
(8 heads x 4 qtiles x 8 groups = 256 steps): QK(gg) + exp(gg) issue at
step gg, AV(gg-3) behind them, the paired l-bursts for groups
(gg-7..gg-4) after that, and a q-tile's PSUM->SBUF copies + DMA-out at
gg-12 once its last AV and l-burst have issued.  No per-qtile or
per-head pipeline drain: the PE stream is uniform from first to last
group (measured ~278us busy vs the 272us QK+AV+l streaming floor at
2.4GHz; exp splits 6/8 ScalarE + 2/8 DVE so neither stalls the PE).

Host pre-transposes Q,K to [D,S] fp16, pre-permutes V to partition-major
[128, NKB, 128] fp16 (so its DMA is linear), and post-applies
out = (out^T / l)^T.
"""

import os

import numpy as np

import concourse.bass as bass
import concourse.tile as tile
from concourse import bacc, mybir
from concourse.bass_utils import run_bass_kernel_spmd

B, H, S, D = 4, 16, 2048, 128
N_CORES = 8
HPC = (B * H) // N_CORES  # heads per core
QT = 512                  # q-tile width (one fp32 PSUM bank)
NQT = S // QT             # 4 q tiles per head
KB = 128                  # kk block (contraction of one matmul)
NKB = S // KB             # 16 kk blocks
GEXP = 2                  # kk blocks batched per exp instruction
NG = NKB // GEXP          # 8 groups per q tile
GPQ = NG                  # groups per q tile
GPH = NQT * NG            # groups per head
TOT = HPC * GPH           # global group count
DVE_GROUPS = (2, 5)       # groups (mod NG) whose exp runs on DVE
EXP_BIAS = -64.0
SCH_A = 128.0 / float(np.log(2.0))          # 184.664...
SCH_B = 16256.0 - 5.5 + EXP_BIAS * SCH_A    # fold bias; -5.5 centers err
F32 = mybir.dt.float32
BF16 = mybir.dt.bfloat16
FP16 = mybir.dt.float16
U16 = mybir.dt.uint16

_NC_CACHE = None


def _build_nc():
    nc = bacc.Bacc("TRN2", target_bir_lowering=False, debug=False)

    qT_d = nc.dram_tensor("qT", [HPC, D, S], FP16, kind="ExternalInput")
    kT_d = nc.dram_tensor("kT", [HPC, D, S], FP16, kind="ExternalInput")
    v_d = nc.dram_tensor("v", [HPC, 128, NKB, D], FP16, kind="ExternalInput")
    oT_d = nc.dram_tensor("outT", [HPC, D, S], F32, kind="ExternalOutput")
    l_d = nc.dram_tensor("lsum", [HPC, NQT, 4, QT], F32, kind="ExternalOutput")

    with tile.TileContext(nc) as tc:
        with (
            tc.tile_pool(name="io", bufs=3) as io,
            tc.tile_pool(name="pexp", bufs=10) as pexp,
            tc.tile_pool(name="osb", bufs=3) as osb_pool,
            tc.tile_pool(name="small", bufs=1) as small,
            tc.tile_pool(name="st", bufs=2, space="PSUM") as st_pool,
            tc.tile_pool(name="acc", bufs=2, space="PSUM") as acc_pool,
        ):
            ones_sb = small.tile([128, 1], BF16)
            nc.vector.memset(ones_sb[:], 1.0)
            bias_sb = small.tile([128, 1], F32)
            nc.vector.memset(bias_sb[:], EXP_BIAS)
            wu_sb = small.tile([128, 256], FP16)
            nc.vector.memset(wu_sb[:], 0.0)
            scr_sb = small.tile([128, 256], BF16)

            # PE pstate warmup while the first DMAs stream in; the dummy
            # activation preloads the exp table off the critical path.
            wu_ps = st_pool.tile([128, GEXP * QT], F32, tag="st")
            nc.tensor.matmul(
                wu_ps[:, :256], wu_sb[:, :128], wu_sb[:],
                start=True, stop=True,
            )
            nc.scalar.activation(
                scr_sb[:],
                wu_sb[:],
                mybir.ActivationFunctionType.Exp,
                bias=bias_sb[:, :],
                scale=1.0,
            )
            for _ in range(10):
                nc.tensor.matmul(
                    wu_ps[:, :256], wu_sb[:, :128], wu_sb[:],
                    start=True, stop=True,
                )

            heads = {}   # hd -> (qT_sb, kT_sb, v_sb)
            accs = {}    # qt_start_gg -> (out_ps, l_ps)
            p_tiles = {} # gg -> p_sb

            for gg in range(TOT + 8):
                if gg < TOT:
                    hd, rem = divmod(gg, GPH)
                    qt, g = divmod(rem, GPQ)

                    if rem == 0:
                        qT_sb = io.tile([128, S], FP16, tag="qT")
                        kT_sb = io.tile([128, S], FP16, tag="kT")
                        v_sb = io.tile([128, NKB, D], FP16, tag="v")
                        heads[hd] = (qT_sb, kT_sb, v_sb)
                        if hd != 0:
                            nc.gpsimd.dma_start(out=v_sb[:], in_=v_d[hd])
                        if hd == 0:
                            # chunked across three queues so QK starts
                            # early and kT keeps ahead of the QK stream
                            nc.sync.dma_start(
                                out=kT_sb[:, :256], in_=kT_d[0, :, :256])
                            nc.scalar.dma_start(
                                out=qT_sb[:, :QT], in_=qT_d[0, :, :QT])
                            nc.sync.dma_start(
                                out=kT_sb[:, 256:QT], in_=kT_d[0, :, 256:QT])
                            nc.gpsimd.dma_start(
                                out=kT_sb[:, QT:2 * QT],
                                in_=kT_d[0, :, QT:2 * QT])
                            nc.sync.dma_start(
                                out=kT_sb[:, 2 * QT:3 * QT],
                                in_=kT_d[0, :, 2 * QT:3 * QT])
                            nc.scalar.dma_start(
                                out=qT_sb[:, QT:], in_=qT_d[0, :, QT:])
                            nc.sync.dma_start(
                                out=kT_sb[:, 3 * QT:], in_=kT_d[0, :, 3 * QT:])
                            nc.gpsimd.dma_start(
                                out=v_sb[:, :4, :], in_=v_d[0, :, :4, :])
                            nc.gpsimd.dma_start(
                                out=v_sb[:, 4:, :], in_=v_d[0, :, 4:, :])
                        else:
                            nc.sync.dma_start(out=qT_sb[:], in_=qT_d[hd])
                            nc.sync.dma_start(out=kT_sb[:], in_=kT_d[hd])
                    else:
                        qT_sb, kT_sb, v_sb = heads[hd]

                    if g == 0:
                        out_ps_new = acc_pool.tile([128, QT], F32, tag="out")
                        l_ps_new = acc_pool.tile([128, QT], F32, tag="l")
                        accs[gg] = (out_ps_new, l_ps_new)

                    # QK for group gg
                    q_sl = qT_sb[:, qt * QT:(qt + 1) * QT]
                    st_ps = st_pool.tile([128, GEXP * QT], F32, tag="st")
                    for j in range(GEXP):
                        kb = g * GEXP + j
                        nc.tensor.matmul(
                            st_ps[:, j * QT:(j + 1) * QT],
                            kT_sb[:, kb * KB:(kb + 1) * KB],
                            q_sl,
                            start=True,
                            stop=True,
                        )
                    # exp for group gg
                    p_sb = pexp.tile([128, GEXP * QT], BF16, tag="p")
                    if gg >= TOT - 2:
                        nc.scalar.activation(
                            p_sb[:, :QT],
                            st_ps[:, :QT],
                            mybir.ActivationFunctionType.Exp,
                            bias=bias_sb[:, :],
                            scale=1.0,
                        )
                        nc.vector.tensor_scalar(
                            p_sb[:, QT:].bitcast(U16),
                            st_ps[:, QT:],
                            SCH_A,
                            SCH_B,
                            mybir.AluOpType.mult,
                            mybir.AluOpType.add,
                        )
                    elif g in DVE_GROUPS:
                        nc.vector.tensor_scalar(
                            p_sb[:].bitcast(U16),
                            st_ps[:],
                            SCH_A,
                            SCH_B,
                            mybir.AluOpType.mult,
                            mybir.AluOpType.add,
                        )
                    else:
                        nc.scalar.activation(
                            p_sb[:],
                            st_ps[:],
                            mybir.ActivationFunctionType.Exp,
                            bias=bias_sb[:, :],
                            scale=1.0,
                        )
                    p_tiles[gg] = p_sb

                # AV for group gg-3
                av = gg - 3
                if 0 <= av < TOT:
                    hd2, rem2 = divmod(av, GPH)
                    g2 = rem2 % GPQ
                    out_ps = accs[av - g2][0]
                    v_sb2 = heads[hd2][2]
                    p_sb2 = p_tiles[av]
                    for j in range(GEXP):
                        kb = g2 * GEXP + j
                        nc.tensor.matmul(
                            out_ps[:],
                            v_sb2[:, kb, :],
                            p_sb2[:, j * QT:(j + 1) * QT],
                            start=(kb == 0),
                            stop=(kb == NKB - 1),
                        )

                # paired l-bursts for groups (gg-7 .. gg-4): two 4-strip
                # bursts back to back halve the burst->QK pipeline refills
                lb = gg - 7
                if lb >= 0 and lb % 4 == 0 and lb < TOT:
                    g3 = lb % GPQ
                    l_ps = accs[lb - g3][1]
                    for half in range(2):
                        r = g3 // 2 + half
                        for j4 in range(4):
                            psrc = p_tiles[lb + 2 * half + j4 // GEXP]
                            nc.tensor.matmul(
                                l_ps[32 * j4:32 * j4 + 1, :],
                                ones_sb[:],
                                psrc[:, (j4 % GEXP) * QT:(j4 % GEXP + 1) * QT],
                                start=(r == 0),
                                stop=(r == NG // 2 - 1),
                                tile_position=(0, 32 * j4),
                            )

                # copies + DMA out for the q tile whose last AV (step
                # qs+10) and last l-burst (step qs+11) have now issued;
                # +14 keeps the copies behind both DVE exps of the
                # following q tile on the in-order DVE queue
                qs = gg - 14
                if qs >= 0 and qs % GPQ == 0:
                    hd4, rem4 = divmod(qs, GPH)
                    qt4 = rem4 // GPQ
                    out_ps, l_ps = accs.pop(qs)
                    out_sb = osb_pool.tile([128, QT], F32, tag="osb")
                    l_sb = osb_pool.tile([128, QT], F32, tag="lsb")
                    # alternate output DMA queues to halve final flush
                    eng_a = nc.gpsimd if qt4 % 2 == 0 else nc.sync
                    eng_b = nc.sync if qt4 % 2 == 0 else nc.gpsimd
                    if qs == TOT - GPQ:
                        # last q tile: chunk copy+DMA to shorten the drain
                        hq = QT // 2
                        nc.vector.tensor_copy(out_sb[:, :hq], out_ps[:, :hq])
                        eng_a.dma_start(
                            out=oT_d[hd4, :, qt4 * QT:qt4 * QT + hq],
                            in_=out_sb[:, :hq],
                        )
                        nc.vector.tensor_copy(out_sb[:, hq:], out_ps[:, hq:])
                        eng_b.dma_start(
                            out=oT_d[hd4, :, qt4 * QT + hq:(qt4 + 1) * QT],
                            in_=out_sb[:, hq:],
                        )
                        nc.vector.tensor_copy(l_sb[:], l_ps[:])
                        eng_a.dma_start(
                            out=l_d[hd4, qt4], in_=l_sb[0:128:32, :]
                        )
                    else:
                        nc.vector.tensor_copy(out_sb[:], out_ps[:])
                        nc.vector.tensor_copy(l_sb[:], l_ps[:])
                        eng_a.dma_start(
                            out=oT_d[hd4, :, qt4 * QT:(qt4 + 1) * QT],
                            in_=out_sb[:],
                        )
                        eng_b.dma_start(
                            out=l_d[hd4, qt4], in_=l_sb[0:128:32, :]
                        )

                if gg - 8 in p_tiles:
                    del p_tiles[gg - 8]
    nc.finalize()
    return nc


def _get_nc():
    global _NC_CACHE
    if _NC_CACHE is None:
        _NC_CACHE = _build_nc()
    return _NC_CACHE


def kernel(q, k, v):
    q = np.asarray(q, dtype=np.float32).reshape(B * H, S, D)
    k = np.asarray(k, dtype=np.float32).reshape(B * H, S, D)
    v = np.asarray(v, dtype=np.float32).reshape(B * H, S, D)

    in_maps = []
    for c in range(N_CORES):
        sl = slice(c * HPC, (c + 1) * HPC)
        # v: [HPC, S, D] -> partition-major [HPC, 128, NKB, D]
        vperm = v[sl].reshape(HPC, NKB, 128, D).transpose(0, 2, 1, 3)
        in_maps.append(
            {
                "qT": np.ascontiguousarray(
                    q[sl].transpose(0, 2, 1)).astype(np.float16),
                "kT": np.ascontiguousarray(
                    k[sl].transpose(0, 2, 1)).astype(np.float16),
                "v": np.ascontiguousarray(vperm).astype(np.float16),
            }
        )

    nc = _get_nc()
    trace = bool(int(os.environ.get("KERNEL_TRACE", "0")))
    res = run_bass_kernel_spmd(
        nc, in_maps, core_ids=list(range(N_CORES)), trace=trace
    )
    if trace:
        print(f"HW exec time: {res.exec_time_ns} ns")
        if res.instructions_and_trace:
            print(f"Trace: {res.instructions_and_trace[1]}")

    out = np.empty((B * H, S, D), dtype=np.float32)
    for c in range(N_CORES):
        oT = res.results[c]["outT"]  # [HPC, D, S]
        l = res.results[c]["lsum"].sum(axis=2).reshape(HPC, S)  # fold strips
        out[c * HPC:(c + 1) * HPC] = oT.transpose(0, 2, 1) / l[:, :, None]
    return out.reshape(B, H, S, D)
